# revision 14
# baseline (speedup 1.0000x reference)
"""GATv2 layer on 8 Trainium2 NeuronCores (Bass/Tile).

Self-contained: takes full inputs, shards internally, returns full output.

Strategy (edge-projection, channel-major): edges bucketed by destination
node; each core owns N/8 destinations, degree-sorted into blocks of 128
(one node per grid column). The host pre-gathers x[src] for every edge
slot into a per-core [128ch, slots] bf16 stream, so the device never does
an indirect gather: a W-stationary matmul projects edge slots straight
into channel-major PSUM chunks (t = W_ext^T xe). s = t + h_dst via a
0-stride rhs matmul.

LeakyReLU identity: LR(x) = x + 0.8 relu(-x) = x - 0.8 min(x, 0), so
  logit_h = sum_c sign(a_c) LR(s'_c) = A^T s' - 0.8 A^T min(s', 0)
with |a| prefolded into W (s' = |a| (.) s) and A the +-1 head-sign
mask. min(s', 0) is a single 4x-mode tensor_scalar_min on DVE; both
logit terms are head-mask matmuls accumulated into the same PSUM
(no LeakyReLU tensor is ever materialized). exp runs full-width.
den/num come from bf16 pair-halvings + a short strided reduce;
num = sum ex*s' - den*h_dst recovers the h_src-weighted sum. Sentinel
slots stream a host-solved x column whose projection makes every head's
logit ~ -6e4 so exp underflows to exactly 0. Softmax max-subtraction is
dropped (mathematically invariant; logits are O(1)).
"""
import os
import sys

for _p in ("/opt/trn_rl_repo", "/root/.axon_site/_ro/trn_rl_repo"):
    if os.path.isdir(_p) and _p not in sys.path:
        sys.path.insert(0, _p)

import numpy as np
import ml_dtypes
import concourse.bass as bass
import concourse.bacc as bacc
import concourse.mybir as mybir
import concourse.tile as tile

P = 128
HEADS = 4
OUT_CH = 32
HC = HEADS * OUT_CH          # 128
EPS_BN = 1e-5
MMCH = 512                   # max matmul out cols (one PSUM bank fp32)
PCHUNK = int(os.environ.get("GAT_PCHUNK", 1024))  # drain/exp tile cols

N_NODES = int(os.environ.get("GAT_N", 100000))
N_CORES = int(os.environ.get("GAT_CORES", 8))
R_CAP = int(os.environ.get("GAT_RCAP", 24))   # multiple of 4
RUN_MODE = os.environ.get("GAT_RUN", "hw")    # hw | sim
TRACE = os.environ.get("GAT_TRACE", "0") == "1"

NODES_PER_CORE = N_NODES // N_CORES
BLOCKS = (NODES_PER_CORE + P - 1) // P
NPAD = BLOCKS * P

f32 = mybir.dt.float32
bf16 = mybir.dt.bfloat16
bfnp = ml_dtypes.bfloat16

LAST_RESULT = {}
_PROGRAM_CACHE = {}


def _host_prep(x, edge_index, W_src, W_dst, att):
    src = edge_index[0].astype(np.int64)
    dst = edge_index[1].astype(np.int64)
    loop = np.arange(N_NODES, dtype=np.int64)
    src2 = np.concatenate([src, loop])
    dst2 = np.concatenate([dst, loop])
    deg = np.bincount(dst2, minlength=N_NODES)
    order = np.argsort(dst2, kind="stable")
    src_sorted = src2[order].astype(np.int64)
    starts = np.zeros(N_NODES + 1, np.int64)
    starts[1:] = np.cumsum(deg)

    # per-core degree-sorted node permutation (pads replicate the core's
    # first node but get a single self-slot)
    perms = np.zeros((N_CORES, NPAD), np.int64)
    is_pad = np.zeros((N_CORES, NPAD), bool)
    for k in range(N_CORES):
        nodes = np.arange(k * NODES_PER_CORE, (k + 1) * NODES_PER_CORE)
        o = np.argsort(-deg[nodes], kind="stable")
        perms[k, :NODES_PER_CORE] = nodes[o]
        perms[k, NODES_PER_CORE:] = nodes[0]
        is_pad[k, NODES_PER_CORE:] = True

    degp = deg[perms]
    degp[is_pad] = 1
    degb = degp.reshape(N_CORES, BLOCKS, P)
    Rb = degb.max(axis=(0, 2)).astype(np.int64)   # uniform across cores
    Rb = (Rb + 1) & ~1                            # even: halving needs rr%2==0

    rounds = []                                   # (block, r_off, rr)
    for b in range(BLOCKS):
        r, roff = int(Rb[b]), 0
        while r > 0:
            rr = min(r, R_CAP)
            rounds.append((b, roff, rr))
            roff += rr
            r -= rr
    tot = sum(rr for _, _, rr in rounds)

    # per-slot source node (SENT = N_NODES -> sentinel row of x_ext),
    # node-major within each round: column = n*rr + r
    SENT = N_NODES
    vals_all = np.full((N_CORES, tot * P), SENT, np.int64)
    off = 0
    for (b, roff, rr) in rounds:
        for k in range(N_CORES):
            nodes = perms[k, b * P:(b + 1) * P]
            pad = is_pad[k, b * P:(b + 1) * P]
            nd = degp[k, b * P:(b + 1) * P]
            j = roff + np.arange(rr)[None, :]                  # [1, rr]
            base = np.where(pad, 0, starts[nodes])[:, None]
            gidx = np.clip(base + j, 0, src_sorted.size - 1)
            v = src_sorted[gidx]                               # [P, rr]
            v = np.where(j < nd[:, None], v, SENT)
            v = np.where(pad[:, None] & (j == 0), nodes[:, None], v)
            vals_all[k, off * P:(off + rr) * P] = v.reshape(-1)
        off += rr

    # --- weights: channel perm (pos att first), |att| prescale ---
    att64 = att.astype(np.float64)
    cperm = np.zeros(HC, np.int64)
    scale = np.zeros(HC, np.float64)
    sbb = []
    for h in range(HEADS):
        pos = np.where(att64[h] > 0)[0]
        neg = np.where(att64[h] <= 0)[0]
        o = np.concatenate([pos, neg])
        sbb.append(len(pos))
        cperm[h * OUT_CH:(h + 1) * OUT_CH] = h * OUT_CH + o
        scale[h * OUT_CH:(h + 1) * OUT_CH] = np.abs(att64[h][o])
    scale = np.maximum(scale, 1e-20)

    def wext(W):
        return (W.astype(np.float64)[:, cperm] * scale[None, :])

    wsrc64 = wext(W_src)
    wdst64 = wext(W_dst)
    wsrc_bf = wsrc64.astype(bfnp)
    wdst_bf = wdst64.astype(bfnp)
    chanscale = (1.0 / scale).astype(np.float32).reshape(HC, 1)

    # signed channel vector (pos channels first per head after cperm)
    sgn = np.zeros(HC, np.float64)
    for h in range(HEADS):
        cs0 = h * OUT_CH
        sgn[cs0:cs0 + sbb[h]] = 1.0
        sgn[cs0 + sbb[h]:cs0 + OUT_CH] = -1.0

    # head-sign mask matrix (+-1), replicated to all 128 output
    # partitions: out channel c' (head h'=(c'//32)): sign(a_c) for
    # channels c of h'.  amat2 = -0.8 * A feeds the min(s,0) term.
    A = np.zeros((HC, HC), np.float64)
    for h in range(HEADS):
        cs0, cs1 = h * OUT_CH, (h + 1) * OUT_CH
        A[cs0:cs0 + sbb[h], cs0:cs1] = 1.0
        A[cs0 + sbb[h]:cs1, cs0:cs1] = -1.0
    A_bf = A.astype(bfnp)
    A2_bf = (-0.8 * A).astype(bfnp)

    # sentinel x column: projects (through the bf16 weights) to
    # t ~ -B*sgn, making every head's logit deeply negative so
    # exp underflows to exactly 0.
    B = 1e4
    Wr = wsrc_bf.astype(np.float64)

    def sent_logit(v):
        t = v.astype(bfnp).astype(np.float64) @ Wr
        u = np.maximum(t, 0.2 * t)
        # a^T LR(t) per head with |a| folded: sum sign * LR(t)
        return max((sgn[h * OUT_CH:(h + 1) * OUT_CH]
                    * u[h * OUT_CH:(h + 1) * OUT_CH]).sum()
                   for h in range(HEADS))

    cands = [np.linalg.solve(Wr.T, -B * sgn)]
    rng = np.random.default_rng(0)
    for _ in range(20):
        jit = sgn + 0.3 * rng.standard_normal(HC)
        v = Wr @ jit
        cands.append(-B * v / (np.abs(Wr.T @ v).mean() + 1e-30))
    xe_sent = None
    for v in cands:
        if sent_logit(v) < -5e3:
            xe_sent = v
            break
    assert xe_sent is not None, "no robust sentinel direction found"

    x_ext = np.concatenate([np.asarray(x, np.float32),
                            xe_sent[None, :].astype(np.float32)], axis=0)
    x_bf = x_ext.astype(bfnp)

    # per-core channel-major edge stream [128, tot*P]
    xeT = np.empty((N_CORES, P, tot * P), bfnp)
    for k in range(N_CORES):
        xeT[k] = x_bf[vals_all[k]].T

    # per-core dst-node stream [128, NPAD]
    xTp = np.empty((N_CORES, P, NPAD), bfnp)
    for k in range(N_CORES):
        xTp[k] = x_bf[perms[k]].T

    ident = np.eye(P, dtype=np.float32)

    return dict(rounds=tuple(rounds), sbb=tuple(sbb), tot=tot,
                perms=perms, cperm=cperm,
                wsrc_bf=np.ascontiguousarray(wsrc_bf),
                wdst_bf=np.ascontiguousarray(wdst_bf),
                A_bf=np.ascontiguousarray(A_bf),
                A2_bf=np.ascontiguousarray(A2_bf),
                cs=chanscale, ident=ident, xeT=xeT, xTp=xTp)


def _build_program(rounds, tot):
    nc = bacc.Bacc("TRN2", target_bir_lowering=False, debug=False,
                   num_devices=N_CORES)
    xeT = nc.dram_tensor("xeT", [P, tot * P], bf16, kind="ExternalInput")
    xTp = nc.dram_tensor("xTp", [P, NPAD], bf16, kind="ExternalInput")
    wsrc = nc.dram_tensor("wsrc", [P, HC], bf16, kind="ExternalInput")
    wdst = nc.dram_tensor("wdst", [P, HC], bf16, kind="ExternalInput")
    amat = nc.dram_tensor("amat", [P, HC], bf16, kind="ExternalInput")
    amat2 = nc.dram_tensor("amat2", [P, HC], bf16, kind="ExternalInput")
    idn = nc.dram_tensor("idn", [P, P], f32, kind="ExternalInput")
    y = nc.dram_tensor("y", [NPAD, HC], f32, kind="ExternalOutput")

    AX = mybir.AxisListType
    OP = mybir.AluOpType
    AF = mybir.ActivationFunctionType

    with tile.TileContext(nc) as tc:
        with (
            tc.tile_pool(name="consts", bufs=1) as cp,
            tc.tile_pool(name="edge", bufs=3) as ep,
            tc.tile_pool(name="work", bufs=3) as wp,
            tc.tile_pool(name="work2", bufs=2) as wp2,
            tc.tile_pool(name="acc", bufs=2) as ap_,
            tc.tile_pool(name="fin", bufs=2) as fp_,
            tc.tile_pool(name="pproj", bufs=2, space="PSUM") as ppj,
            tc.tile_pool(name="plogit", bufs=2, space="PSUM") as plg,
        ):
            wsrc_t = cp.tile([P, HC], bf16)
            nc.sync.dma_start(out=wsrc_t[:], in_=wsrc[:])
            wdst_t = cp.tile([P, HC], bf16)
            nc.sync.dma_start(out=wdst_t[:], in_=wdst[:])
            amat_t = cp.tile([P, HC], bf16)
            nc.sync.dma_start(out=amat_t[:], in_=amat[:])
            amat2_t = cp.tile([P, HC], bf16)
            nc.sync.dma_start(out=amat2_t[:], in_=amat2[:])
            idn_t = cp.tile([P, P], f32)
            nc.sync.dma_start(out=idn_t[:], in_=idn[:])
            xtp_t = cp.tile([P, NPAD], bf16)
            nc.sync.dma_start(out=xtp_t[:], in_=xTp[:])

            # ---- h_dst projection (channel-major, resident) ----
            hd_cm = cp.tile([P, NPAD], f32)
            for c0 in range(0, NPAD, MMCH):
                cw = min(MMCH, NPAD - c0)
                ps = ppj.tile([P, PCHUNK], f32, space="PSUM", tag="pp")
                nc.tensor.matmul(out=ps[:, :cw], lhsT=wdst_t[:],
                                 rhs=xtp_t[:, c0:c0 + cw],
                                 start=True, stop=True)
                nc.scalar.copy(out=hd_cm[:, c0:c0 + cw], in_=ps[:, :cw])

            # ---- edge phase ----
            n_in_block = {}
            for b, _, _ in rounds:
                n_in_block[b] = n_in_block.get(b, 0) + 1
            done_in_block = 0
            cur_b = -1
            nd_t = None
            off = 0

            for (b, roff, rr) in rounds:
                first = b != cur_b
                if first:
                    cur_b = b
                    done_in_block = 0
                    nd_t = ap_.tile([P, 2 * P], f32, tag="nd")
                done_in_block += 1
                last = done_in_block == n_in_block[b]

                ns = rr * P

                xet = ep.tile([P, R_CAP * P], bf16, tag="xet")
                nc.sync.dma_start(out=xet[:, :ns],
                                  in_=xeT[:, off * P:(off + rr) * P])
                off += rr

                # projection: s = Wsrc^T xe + Wdst^T xd (0-stride rhs
                # replicates each dst column rr times). Matmuls cannot
                # cross PSUM bank boundaries, so each 512-col bank holds
                # kb = 512//rr whole nodes (tail of the bank unwritten);
                # the ACT drain reads a [512-strided, kb*rr] AP so SBUF
                # stays dense.
                s_t = wp.tile([P, R_CAP * P], bf16, tag="s")
                kb = MMCH // rr             # nodes per PSUM bank
                n0 = 0
                while n0 < P:
                    ps = ppj.tile([P, PCHUNK], f32, space="PSUM", tag="pp")
                    t0 = n0 * rr
                    slices = []
                    for bank in range(PCHUNK // MMCH):
                        k = min(kb, P - n0)
                        if k == 0:
                            break
                        slices.append((n0, bank * MMCH, k))
                        n0 += k
                    for (sn0, soff, k) in slices:
                        nc.tensor.matmul(
                            out=ps[:, soff:soff + k * rr], lhsT=wsrc_t[:],
                            rhs=xet[:, sn0 * rr:(sn0 + k) * rr],
                            start=True, stop=False)
                    for (sn0, soff, k) in slices:
                        a = xtp_t[:, b * P + sn0:b * P + sn0 + k]
                        xdv = bass.AP(a.tensor, a.offset,
                                      [list(a.ap[0]), list(a.ap[-1]), [0, rr]])
                        nc.tensor.matmul(
                            out=ps[:, soff:soff + k * rr], lhsT=wdst_t[:],
                            rhs=xdv, start=False, stop=True)
                    if len(slices) == 2 and slices[0][2] == slices[1][2]:
                        # uniform banks: one strided drain
                        k = slices[0][2]
                        pv = ps[:]
                        src = bass.AP(pv.tensor, pv.offset,
                                      [list(pv.ap[0]), [MMCH, 2], [1, k * rr]])
                        dst = s_t[:, t0:t0 + 2 * k * rr].rearrange(
                            "p (b c) -> p b c", b=2)
                        nc.scalar.copy(out=dst, in_=src)
                    else:
                        for (sn0, soff, k) in slices:
                            nc.scalar.copy(
                                out=s_t[:, sn0 * rr:(sn0 + k) * rr],
                                in_=ps[:, soff:soff + k * rr])

                # min(s', 0) in one 4x-mode pass
                m_t = wp.tile([P, R_CAP * P], bf16, tag="m")
                nc.vector.tensor_scalar(out=m_t[:, :ns], in0=s_t[:, :ns],
                                        scalar1=0.0, scalar2=None,
                                        op0=OP.min)

                # logits: LR(x) = x - 0.8 min(x,0), so logit PSUM =
                # A^T s' + (-0.8 A)^T min(s',0); ACT exp drains each
                # PCHUNK tile in one pass.
                em_t = wp.tile([P, 2 * R_CAP * P], bf16, tag="em")
                for c0 in range(0, ns, PCHUNK):
                    cw2 = min(PCHUNK, ns - c0)
                    pl = plg.tile([P, PCHUNK], f32, space="PSUM", tag="pl")
                    for sc in range(0, cw2, MMCH):
                        sw = min(MMCH, cw2 - sc)
                        nc.tensor.matmul(out=pl[:, sc:sc + sw],
                                         lhsT=amat_t[:],
                                         rhs=s_t[:, c0 + sc:c0 + sc + sw],
                                         start=True, stop=False)
                    for sc in range(0, cw2, MMCH):
                        sw = min(MMCH, cw2 - sc)
                        nc.tensor.matmul(out=pl[:, sc:sc + sw],
                                         lhsT=amat2_t[:],
                                         rhs=m_t[:, c0 + sc:c0 + sc + sw],
                                         start=False, stop=True)
                    nc.scalar.activation(out=em_t[:, c0:c0 + cw2],
                                         in_=pl[:, :cw2], func=AF.Exp)

                nc.vector.tensor_tensor(out=em_t[:, ns:2 * ns],
                                        in0=em_t[:, :ns],
                                        in1=s_t[:, :ns], op=OP.mult)

                # segmented sum over r: bf16 pair-halvings (2x TT) then a
                # short strided reduce (reduce has no fast modes).
                target = nd_t if first else wp2.tile([P, 2 * P], f32,
                                                     tag="ndt")
                tv = target[:].rearrange("p (g n) -> p g n", g=2)
                tv1 = target[:].rearrange("p (g n r) -> p g n r", g=2, r=1)
                emv0 = em_t[:, :2 * ns].rearrange("p (g n r) -> p g n r",
                                                  g=2, r=rr)
                if rr == 2:
                    nc.vector.tensor_tensor(out=tv1, in0=emv0[:, :, :, :1],
                                            in1=emv0[:, :, :, 1:],
                                            op=OP.add)
                else:
                    h1 = rr // 2
                    em2 = wp2.tile([P, R_CAP * P], bf16, tag="em2")
                    e2v = em2[:, :2 * P * h1].rearrange(
                        "p (g n r) -> p g n r", g=2, r=h1)
                    nc.vector.tensor_tensor(out=e2v, in0=emv0[:, :, :, :h1],
                                            in1=emv0[:, :, :, h1:],
                                            op=OP.add)
                    if rr % 4 == 0:
                        h2 = rr // 4
                        if h2 == 1:
                            nc.vector.tensor_tensor(out=tv1,
                                                    in0=e2v[:, :, :, :1],
                                                    in1=e2v[:, :, :, 1:],
                                                    op=OP.add)
                        else:
                            nc.vector.tensor_tensor(out=e2v[:, :, :, :h2],
                                                    in0=e2v[:, :, :, :h2],
                                                    in1=e2v[:, :, :, h2:2 * h2],
                                                    op=OP.add)
                            nc.vector.reduce_sum(out=tv,
                                                 in_=e2v[:, :, :, :h2],
                                                 axis=AX.X)
                    else:
                        nc.vector.reduce_sum(out=tv, in_=e2v, axis=AX.X)
                if not first:
                    nc.vector.tensor_tensor(out=nd_t[:], in0=nd_t[:],
                                            in1=target[:], op=OP.add)

                if last:
                    # t2 = num - den*hd (GpSimd); transpose t2, den (PE,
                    # sharing the logit PSUM ring); y_nm = t2_nm *
                    # recip(den_nm) broadcast per head
                    hd_b = hd_cm[:, b * P:(b + 1) * P]
                    t1 = fp_.tile([P, P], f32, tag="t1")
                    nc.gpsimd.tensor_tensor(out=t1[:], in0=nd_t[:, :P],
                                            in1=hd_b, op=OP.mult)
                    nc.gpsimd.tensor_tensor(out=t1[:], in0=nd_t[:, P:],
                                            in1=t1[:], op=OP.subtract)
                    pt = plg.tile([P, PCHUNK], f32, space="PSUM", tag="pl")
                    nc.tensor.transpose(out=pt[:, :P], in_=t1[:],
                                        identity=idn_t[:])
                    nc.tensor.transpose(out=pt[:, P:2 * P], in_=nd_t[:, :P],
                                        identity=idn_t[:])
                    rden = fp_.tile([P, HEADS], f32, tag="rden")
                    dh = pt[:, P:2 * P]
                    dhv = bass.AP(dh.tensor, dh.offset,
                                  [list(dh.ap[0]), [OUT_CH, HEADS]])
                    nc.vector.reciprocal(out=rden[:], in_=dhv)
                    ynm = fp_.tile([P, P], f32, tag="ynm")
                    r = rden[:]
                    rv = bass.AP(r.tensor, r.offset,
                                 [list(r.ap[0]), [1, HEADS], [0, OUT_CH]])
                    yv = ynm[:].rearrange("p (h c) -> p h c", c=OUT_CH)
                    pv2 = pt[:, :P].rearrange("p (h c) -> p h c", c=OUT_CH)
                    nc.vector.tensor_tensor(out=yv, in0=pv2, in1=rv,
                                            op=OP.mult)
                    nc.sync.dma_start(out=y[b * P:(b + 1) * P, :],
                                      in_=ynm[:])

    nc.compile()
    return nc


def _run(nc, in_maps):
    if RUN_MODE == "sim":
        from concourse import bass_interp
        assert N_CORES == 1
        sim = bass_interp.CoreSim(nc)
        for name, arr in in_maps[0].items():
            sim.tensor(name)[:] = arr
        sim.simulate()
        return [{"y": np.array(sim.tensor("y"))}]
    from concourse.bass_utils import run_bass_kernel_spmd
    if TRACE:
        try:
            import axon_prof  # noqa: F401  (registers NTFF hook)
        except Exception:
            pass
    res = run_bass_kernel_spmd(nc, in_maps, list(range(N_CORES)), trace=TRACE)
    LAST_RESULT["exec_time_ns"] = res.exec_time_ns
    LAST_RESULT["res"] = res
    return res.results


def kernel(x, edge_index, W_src, W_dst, att, bias, bn_gamma, bn_beta):
    x = np.asarray(x, np.float32)
    edge_index = np.asarray(edge_index)
    prep = _host_prep(x, edge_index, np.asarray(W_src), np.asarray(W_dst),
                      np.asarray(att))

    key = (prep["rounds"],)
    if key not in _PROGRAM_CACHE:
        _PROGRAM_CACHE[key] = _build_program(prep["rounds"], prep["tot"])
    nc = _PROGRAM_CACHE[key]

    in_maps = []
    for k in range(N_CORES):
        in_maps.append({
            "xeT": prep["xeT"][k],
            "xTp": prep["xTp"][k],
            "wsrc": prep["wsrc_bf"],
            "wdst": prep["wdst_bf"],
            "amat": prep["A_bf"],
            "amat2": prep["A2_bf"],
            "idn": prep["ident"],
        })
    results = _run(nc, in_maps)

    out = np.zeros((N_NODES, HC), np.float32)
    for k in range(N_CORES):
        yk = np.asarray(results[k]["y"])[:NODES_PER_CORE]
        yk = yk * prep["cs"].ravel()[None, :]
        out[np.ix_(prep["perms"][k][:NODES_PER_CORE], prep["cperm"])] = yk

    # bias + BatchNorm (batch stats) + LeakyReLU(0.02) epilogue
    out = out + np.asarray(bias, np.float32)[None, :]
    mean = out.mean(axis=0)
    var = out.var(axis=0)
    yv = (np.asarray(bn_gamma, np.float32) * (out - mean)
          / np.sqrt(var + EPS_BN) + np.asarray(bn_beta, np.float32))
    return np.where(yv > 0, yv, 0.02 * yv).astype(np.float32)


# revision 16
# speedup vs baseline: 1.1117x; 1.1117x over previous
"""GATv2 layer on 8 Trainium2 NeuronCores (Bass/Tile).

Self-contained: takes full inputs, shards internally, returns full output.

Strategy (edge-projection, channel-major): edges bucketed by destination
node; each core owns N/8 destinations, degree-sorted into blocks of 128
(one node per grid column). The host pre-gathers x[src] for every edge
slot into a per-core [128ch, slots] bf16 stream, so the device never does
an indirect gather: a W-stationary matmul projects edge slots straight
into channel-major PSUM chunks (t = W_ext^T xe). s = t + h_dst via a
0-stride rhs matmul.

LeakyReLU identity: LR(x) = x + 0.8 relu(-x) = x - 0.8 min(x, 0), so
  logit_h = sum_c sign(a_c) LR(s'_c) = A^T s' - 0.8 A^T min(s', 0)
with |a| prefolded into W (s' = |a| (.) s) and A the +-1 head-sign
mask. min(s', 0) is a single 4x-mode tensor_scalar_min on DVE; both
logit terms are head-mask matmuls accumulated into the same PSUM
(no LeakyReLU tensor is ever materialized). exp runs full-width.
den/num come from bf16 pair-halvings + a short strided reduce;
num = sum ex*s' - den*h_dst recovers the h_src-weighted sum. Sentinel
slots stream a host-solved x column whose projection makes every head's
logit ~ -6e4 so exp underflows to exactly 0. Softmax max-subtraction is
dropped (mathematically invariant; logits are O(1)).
"""
import os
import sys

for _p in ("/opt/trn_rl_repo", "/root/.axon_site/_ro/trn_rl_repo"):
    if os.path.isdir(_p) and _p not in sys.path:
        sys.path.insert(0, _p)

import numpy as np
import ml_dtypes
import concourse.bass as bass
import concourse.bacc as bacc
import concourse.mybir as mybir
import concourse.tile as tile

P = 128
HEADS = 4
OUT_CH = 32
HC = HEADS * OUT_CH          # 128
EPS_BN = 1e-5
MMCH = 512                   # max matmul out cols (one PSUM bank fp32)
PCHUNK = int(os.environ.get("GAT_PCHUNK", 1024))  # drain/exp tile cols

N_NODES = int(os.environ.get("GAT_N", 100000))
N_CORES = int(os.environ.get("GAT_CORES", 8))
R_CAP = int(os.environ.get("GAT_RCAP", 24))   # multiple of 4
RUN_MODE = os.environ.get("GAT_RUN", "hw")    # hw | sim
TRACE = os.environ.get("GAT_TRACE", "0") == "1"

NODES_PER_CORE = N_NODES // N_CORES
BLOCKS = (NODES_PER_CORE + P - 1) // P
NPAD = BLOCKS * P

f32 = mybir.dt.float32
bf16 = mybir.dt.bfloat16
bfnp = ml_dtypes.bfloat16

LAST_RESULT = {}
_PROGRAM_CACHE = {}

if os.environ.get("GAT_LDWOPT", "0") == "1":
    # walrus pins --enable-ldw-opt=false; flip it so consecutive
    # same-weights matmuls skip the LDWEIGHTS reload.
    import concourse.bass_utils as _bu

    if not getattr(_bu, "_gat_ldwopt_patched", False):
        _orig_rc = _bu.run_command

        def _rc(cmd, **kw):
            cmd = [c.replace("--enable-ldw-opt=false", "--enable-ldw-opt=true")
                   if isinstance(c, str) else c for c in cmd]
            return _orig_rc(cmd, **kw)

        _bu.run_command = _rc
        _bu._gat_ldwopt_patched = True


def _host_prep(x, edge_index, W_src, W_dst, att):
    src = edge_index[0].astype(np.int64)
    dst = edge_index[1].astype(np.int64)
    loop = np.arange(N_NODES, dtype=np.int64)
    src2 = np.concatenate([src, loop])
    dst2 = np.concatenate([dst, loop])
    deg = np.bincount(dst2, minlength=N_NODES)
    order = np.argsort(dst2, kind="stable")
    src_sorted = src2[order].astype(np.int64)
    starts = np.zeros(N_NODES + 1, np.int64)
    starts[1:] = np.cumsum(deg)

    # per-core degree-sorted node permutation (pads replicate the core's
    # first node but get a single self-slot)
    perms = np.zeros((N_CORES, NPAD), np.int64)
    is_pad = np.zeros((N_CORES, NPAD), bool)
    for k in range(N_CORES):
        nodes = np.arange(k * NODES_PER_CORE, (k + 1) * NODES_PER_CORE)
        o = np.argsort(-deg[nodes], kind="stable")
        perms[k, :NODES_PER_CORE] = nodes[o]
        perms[k, NODES_PER_CORE:] = nodes[0]
        is_pad[k, NODES_PER_CORE:] = True

    degp = deg[perms]
    degp[is_pad] = 1
    degb = degp.reshape(N_CORES, BLOCKS, P)
    Rb = degb.max(axis=(0, 2)).astype(np.int64)   # uniform across cores
    Rb = (Rb + 1) & ~1                            # even: halving needs rr%2==0

    rounds = []                                   # (block, r_off, rr)
    for b in range(BLOCKS):
        r, roff = int(Rb[b]), 0
        while r > 0:
            rr = min(r, R_CAP)
            rounds.append((b, roff, rr))
            roff += rr
            r -= rr
    tot = sum(rr for _, _, rr in rounds)

    # per-slot source node (SENT = N_NODES -> sentinel row of x_ext),
    # node-major within each round: column = n*rr + r
    SENT = N_NODES
    vals_all = np.full((N_CORES, tot * P), SENT, np.int64)
    off = 0
    for (b, roff, rr) in rounds:
        for k in range(N_CORES):
            nodes = perms[k, b * P:(b + 1) * P]
            pad = is_pad[k, b * P:(b + 1) * P]
            nd = degp[k, b * P:(b + 1) * P]
            j = roff + np.arange(rr)[None, :]                  # [1, rr]
            base = np.where(pad, 0, starts[nodes])[:, None]
            gidx = np.clip(base + j, 0, src_sorted.size - 1)
            v = src_sorted[gidx]                               # [P, rr]
            v = np.where(j < nd[:, None], v, SENT)
            v = np.where(pad[:, None] & (j == 0), nodes[:, None], v)
            vals_all[k, off * P:(off + rr) * P] = v.reshape(-1)
        off += rr

    # --- weights: channel perm (pos att first), |att| prescale ---
    att64 = att.astype(np.float64)
    cperm = np.zeros(HC, np.int64)
    scale = np.zeros(HC, np.float64)
    sbb = []
    for h in range(HEADS):
        pos = np.where(att64[h] > 0)[0]
        neg = np.where(att64[h] <= 0)[0]
        o = np.concatenate([pos, neg])
        sbb.append(len(pos))
        cperm[h * OUT_CH:(h + 1) * OUT_CH] = h * OUT_CH + o
        scale[h * OUT_CH:(h + 1) * OUT_CH] = np.abs(att64[h][o])
    scale = np.maximum(scale, 1e-20)

    def wext(W):
        return (W.astype(np.float64)[:, cperm] * scale[None, :])

    wsrc64 = wext(W_src)
    wdst64 = wext(W_dst)
    wsrc_bf = wsrc64.astype(bfnp)
    wdst_bf = wdst64.astype(bfnp)
    chanscale = (1.0 / scale).astype(np.float32).reshape(HC, 1)

    # signed channel vector (pos channels first per head after cperm)
    sgn = np.zeros(HC, np.float64)
    for h in range(HEADS):
        cs0 = h * OUT_CH
        sgn[cs0:cs0 + sbb[h]] = 1.0
        sgn[cs0 + sbb[h]:cs0 + OUT_CH] = -1.0

    # head-sign mask matrix (+-1), replicated to all 128 output
    # partitions: out channel c' (head h'=(c'//32)): sign(a_c) for
    # channels c of h'.  amat2 = -0.8 * A feeds the min(s,0) term.
    A = np.zeros((HC, HC), np.float64)
    for h in range(HEADS):
        cs0, cs1 = h * OUT_CH, (h + 1) * OUT_CH
        A[cs0:cs0 + sbb[h], cs0:cs1] = 1.0
        A[cs0 + sbb[h]:cs1, cs0:cs1] = -1.0
    A_bf = A.astype(bfnp)
    A2_bf = (-0.8 * A).astype(bfnp)

    # sentinel x column: projects (through the bf16 weights) to
    # t ~ -B*sgn, making every head's logit deeply negative so
    # exp underflows to exactly 0.
    B = 1e4
    Wr = wsrc_bf.astype(np.float64)

    def sent_logit(v):
        t = v.astype(bfnp).astype(np.float64) @ Wr
        u = np.maximum(t, 0.2 * t)
        # a^T LR(t) per head with |a| folded: sum sign * LR(t)
        return max((sgn[h * OUT_CH:(h + 1) * OUT_CH]
                    * u[h * OUT_CH:(h + 1) * OUT_CH]).sum()
                   for h in range(HEADS))

    cands = [np.linalg.solve(Wr.T, -B * sgn)]
    rng = np.random.default_rng(0)
    for _ in range(20):
        jit = sgn + 0.3 * rng.standard_normal(HC)
        v = Wr @ jit
        cands.append(-B * v / (np.abs(Wr.T @ v).mean() + 1e-30))
    xe_sent = None
    for v in cands:
        if sent_logit(v) < -5e3:
            xe_sent = v
            break
    assert xe_sent is not None, "no robust sentinel direction found"

    x_ext = np.concatenate([np.asarray(x, np.float32),
                            xe_sent[None, :].astype(np.float32)], axis=0)
    x_bf = x_ext.astype(bfnp)

    # per-core channel-major edge stream [128, tot*P]
    xeT = np.empty((N_CORES, P, tot * P), bfnp)
    for k in range(N_CORES):
        xeT[k] = x_bf[vals_all[k]].T

    # per-core dst-node stream [128, NPAD]
    xTp = np.empty((N_CORES, P, NPAD), bfnp)
    for k in range(N_CORES):
        xTp[k] = x_bf[perms[k]].T

    ident = np.eye(P, dtype=np.float32)

    return dict(rounds=tuple(rounds), sbb=tuple(sbb), tot=tot,
                perms=perms, cperm=cperm,
                wsrc_bf=np.ascontiguousarray(wsrc_bf),
                wdst_bf=np.ascontiguousarray(wdst_bf),
                A_bf=np.ascontiguousarray(A_bf),
                A2_bf=np.ascontiguousarray(A2_bf),
                cs=chanscale, ident=ident, xeT=xeT, xTp=xTp)


def _build_program(rounds, tot):
    nc = bacc.Bacc("TRN2", target_bir_lowering=False, debug=False,
                   num_devices=N_CORES)
    xeT = nc.dram_tensor("xeT", [P, tot * P], bf16, kind="ExternalInput")
    xTp = nc.dram_tensor("xTp", [P, NPAD], bf16, kind="ExternalInput")
    wsrc = nc.dram_tensor("wsrc", [P, HC], bf16, kind="ExternalInput")
    wdst = nc.dram_tensor("wdst", [P, HC], bf16, kind="ExternalInput")
    amat = nc.dram_tensor("amat", [P, HC], bf16, kind="ExternalInput")
    amat2 = nc.dram_tensor("amat2", [P, HC], bf16, kind="ExternalInput")
    idn = nc.dram_tensor("idn", [P, P], f32, kind="ExternalInput")
    y = nc.dram_tensor("y", [NPAD, HC], f32, kind="ExternalOutput")

    AX = mybir.AxisListType
    OP = mybir.AluOpType
    AF = mybir.ActivationFunctionType

    with tile.TileContext(nc) as tc:
        with (
            tc.tile_pool(name="consts", bufs=1) as cp,
            tc.tile_pool(name="edge", bufs=3) as ep,
            tc.tile_pool(name="work", bufs=3) as wp,
            tc.tile_pool(name="work2", bufs=2) as wp2,
            tc.tile_pool(name="acc", bufs=2) as ap_,
            tc.tile_pool(name="fin", bufs=2) as fp_,
            tc.tile_pool(name="pproj", bufs=2, space="PSUM") as ppj,
            tc.tile_pool(name="plogit", bufs=2, space="PSUM") as plg,
        ):
            wsrc_t = cp.tile([P, HC], bf16)
            nc.sync.dma_start(out=wsrc_t[:], in_=wsrc[:])
            wdst_t = cp.tile([P, HC], bf16)
            nc.sync.dma_start(out=wdst_t[:], in_=wdst[:])
            amat_t = cp.tile([P, HC], bf16)
            nc.sync.dma_start(out=amat_t[:], in_=amat[:])
            amat2_t = cp.tile([P, HC], bf16)
            nc.sync.dma_start(out=amat2_t[:], in_=amat2[:])
            idn_t = cp.tile([P, P], f32)
            nc.sync.dma_start(out=idn_t[:], in_=idn[:])
            xtp_t = cp.tile([P, NPAD], bf16)
            nc.sync.dma_start(out=xtp_t[:], in_=xTp[:])

            # ---- h_dst projection (channel-major, resident) ----
            hd_cm = cp.tile([P, NPAD], f32)
            for c0 in range(0, NPAD, MMCH):
                cw = min(MMCH, NPAD - c0)
                ps = ppj.tile([P, PCHUNK], f32, space="PSUM", tag="pp")
                nc.tensor.matmul(out=ps[:, :cw], lhsT=wdst_t[:],
                                 rhs=xtp_t[:, c0:c0 + cw],
                                 start=True, stop=True)
                nc.scalar.copy(out=hd_cm[:, c0:c0 + cw], in_=ps[:, :cw])

            # ---- edge phase ----
            n_in_block = {}
            for b, _, _ in rounds:
                n_in_block[b] = n_in_block.get(b, 0) + 1
            done_in_block = 0
            cur_b = -1
            nd_t = None
            off = 0

            for (b, roff, rr) in rounds:
                first = b != cur_b
                if first:
                    cur_b = b
                    done_in_block = 0
                    nd_t = ap_.tile([P, 2 * P], f32, tag="nd")
                done_in_block += 1
                last = done_in_block == n_in_block[b]

                ns = rr * P

                xet = ep.tile([P, R_CAP * P], bf16, tag="xet")
                nc.sync.dma_start(out=xet[:, :ns],
                                  in_=xeT[:, off * P:(off + rr) * P])
                off += rr

                # Per-tile pipeline (proj -> drain -> min -> logits ->
                # exp -> mult), so every engine starts as soon as its
                # slice of the round is ready.  s = Wsrc^T xe + Wdst^T xd
                # (0-stride rhs replicates each dst column rr times).
                # Matmuls cannot cross PSUM bank boundaries, so each
                # 512-col proj bank holds kb = 512//rr whole nodes (bank
                # tail unwritten); the ACT drain reads a strided AP so
                # SBUF stays dense.  Logit tiles are dense 512-sliced.
                s_t = wp.tile([P, R_CAP * P], bf16, tag="s")
                m_t = wp.tile([P, R_CAP * P], bf16, tag="m")
                em_t = wp.tile([P, 2 * R_CAP * P], bf16, tag="em")
                kb = MMCH // rr             # nodes per PSUM bank
                n0 = 0
                while n0 < P:
                    ps = ppj.tile([P, PCHUNK], f32, space="PSUM", tag="pp")
                    t0 = n0 * rr
                    slices = []
                    for bank in range(PCHUNK // MMCH):
                        k = min(kb, P - n0)
                        if k == 0:
                            break
                        slices.append((n0, bank * MMCH, k))
                        n0 += k
                    for (sn0, soff, k) in slices:
                        nc.tensor.matmul(
                            out=ps[:, soff:soff + k * rr], lhsT=wsrc_t[:],
                            rhs=xet[:, sn0 * rr:(sn0 + k) * rr],
                            start=True, stop=False)
                    for (sn0, soff, k) in slices:
                        a = xtp_t[:, b * P + sn0:b * P + sn0 + k]
                        xdv = bass.AP(a.tensor, a.offset,
                                      [list(a.ap[0]), list(a.ap[-1]), [0, rr]])
                        nc.tensor.matmul(
                            out=ps[:, soff:soff + k * rr], lhsT=wdst_t[:],
                            rhs=xdv, start=False, stop=True)
                    used = n0 * rr - t0
                    if len(slices) == 2 and slices[0][2] == slices[1][2]:
                        # uniform banks: one strided drain
                        k = slices[0][2]
                        pv = ps[:]
                        src = bass.AP(pv.tensor, pv.offset,
                                      [list(pv.ap[0]), [MMCH, 2], [1, k * rr]])
                        dst = s_t[:, t0:t0 + 2 * k * rr].rearrange(
                            "p (b c) -> p b c", b=2)
                        nc.scalar.copy(out=dst, in_=src)
                    else:
                        for (sn0, soff, k) in slices:
                            nc.scalar.copy(
                                out=s_t[:, sn0 * rr:(sn0 + k) * rr],
                                in_=ps[:, soff:soff + k * rr])

                    # min(s', 0) for this tile span (4x mode)
                    nc.vector.tensor_scalar(out=m_t[:, t0:t0 + used],
                                            in0=s_t[:, t0:t0 + used],
                                            scalar1=0.0, scalar2=None,
                                            op0=OP.min)

                    # logits: LR(x) = x - 0.8 min(x,0), so logit PSUM =
                    # A^T s' + (-0.8 A)^T min(s',0); exp drains the tile
                    # in one ACT pass; mult forms ex*s for this span.
                    pl = plg.tile([P, PCHUNK], f32, space="PSUM", tag="pl")
                    for sc in range(0, used, MMCH):
                        sw = min(MMCH, used - sc)
                        nc.tensor.matmul(out=pl[:, sc:sc + sw],
                                         lhsT=amat_t[:],
                                         rhs=s_t[:, t0 + sc:t0 + sc + sw],
                                         start=True, stop=False)
                    for sc in range(0, used, MMCH):
                        sw = min(MMCH, used - sc)
                        nc.tensor.matmul(out=pl[:, sc:sc + sw],
                                         lhsT=amat2_t[:],
                                         rhs=m_t[:, t0 + sc:t0 + sc + sw],
                                         start=False, stop=True)
                    nc.scalar.activation(out=em_t[:, t0:t0 + used],
                                         in_=pl[:, :used], func=AF.Exp)
                    nc.vector.tensor_tensor(out=em_t[:, ns + t0:ns + t0 + used],
                                            in0=em_t[:, t0:t0 + used],
                                            in1=s_t[:, t0:t0 + used],
                                            op=OP.mult)

                # segmented sum over r: bf16 pair-halvings (2x TT) then a
                # short strided reduce (reduce has no fast modes).
                target = nd_t if first else wp2.tile([P, 2 * P], f32,
                                                     tag="ndt")
                tv = target[:].rearrange("p (g n) -> p g n", g=2)
                tv1 = target[:].rearrange("p (g n r) -> p g n r", g=2, r=1)
                emv0 = em_t[:, :2 * ns].rearrange("p (g n r) -> p g n r",
                                                  g=2, r=rr)
                if rr == 2:
                    nc.vector.tensor_tensor(out=tv1, in0=emv0[:, :, :, :1],
                                            in1=emv0[:, :, :, 1:],
                                            op=OP.add)
                else:
                    h1 = rr // 2
                    em2 = wp2.tile([P, R_CAP * P], bf16, tag="em2")
                    e2v = em2[:, :2 * P * h1].rearrange(
                        "p (g n r) -> p g n r", g=2, r=h1)
                    nc.vector.tensor_tensor(out=e2v, in0=emv0[:, :, :, :h1],
                                            in1=emv0[:, :, :, h1:],
                                            op=OP.add)
                    if rr % 4 == 0:
                        h2 = rr // 4
                        if h2 == 1:
                            nc.vector.tensor_tensor(out=tv1,
                                                    in0=e2v[:, :, :, :1],
                                                    in1=e2v[:, :, :, 1:],
                                                    op=OP.add)
                        else:
                            nc.vector.tensor_tensor(out=e2v[:, :, :, :h2],
                                                    in0=e2v[:, :, :, :h2],
                                                    in1=e2v[:, :, :, h2:2 * h2],
                                                    op=OP.add)
                            nc.vector.reduce_sum(out=tv,
                                                 in_=e2v[:, :, :, :h2],
                                                 axis=AX.X)
                    else:
                        nc.vector.reduce_sum(out=tv, in_=e2v, axis=AX.X)
                if not first:
                    nc.vector.tensor_tensor(out=nd_t[:], in0=nd_t[:],
                                            in1=target[:], op=OP.add)

                if last:
                    # t2 = num - den*hd (GpSimd); transpose t2, den (PE,
                    # sharing the logit PSUM ring); y_nm = t2_nm *
                    # recip(den_nm) broadcast per head
                    hd_b = hd_cm[:, b * P:(b + 1) * P]
                    t1 = fp_.tile([P, P], f32, tag="t1")
                    nc.gpsimd.tensor_tensor(out=t1[:], in0=nd_t[:, :P],
                                            in1=hd_b, op=OP.mult)
                    nc.gpsimd.tensor_tensor(out=t1[:], in0=nd_t[:, P:],
                                            in1=t1[:], op=OP.subtract)
                    pt = plg.tile([P, PCHUNK], f32, space="PSUM", tag="pl")
                    nc.tensor.transpose(out=pt[:, :P], in_=t1[:],
                                        identity=idn_t[:])
                    nc.tensor.transpose(out=pt[:, P:2 * P], in_=nd_t[:, :P],
                                        identity=idn_t[:])
                    rden = fp_.tile([P, HEADS], f32, tag="rden")
                    dh = pt[:, P:2 * P]
                    dhv = bass.AP(dh.tensor, dh.offset,
                                  [list(dh.ap[0]), [OUT_CH, HEADS]])
                    nc.vector.reciprocal(out=rden[:], in_=dhv)
                    ynm = fp_.tile([P, P], f32, tag="ynm")
                    r = rden[:]
                    rv = bass.AP(r.tensor, r.offset,
                                 [list(r.ap[0]), [1, HEADS], [0, OUT_CH]])
                    yv = ynm[:].rearrange("p (h c) -> p h c", c=OUT_CH)
                    pv2 = pt[:, :P].rearrange("p (h c) -> p h c", c=OUT_CH)
                    nc.vector.tensor_tensor(out=yv, in0=pv2, in1=rv,
                                            op=OP.mult)
                    nc.sync.dma_start(out=y[b * P:(b + 1) * P, :],
                                      in_=ynm[:])

    nc.compile()
    return nc


def _run(nc, in_maps):
    if RUN_MODE == "sim":
        from concourse import bass_interp
        assert N_CORES == 1
        sim = bass_interp.CoreSim(nc)
        for name, arr in in_maps[0].items():
            sim.tensor(name)[:] = arr
        sim.simulate()
        return [{"y": np.array(sim.tensor("y"))}]
    from concourse.bass_utils import run_bass_kernel_spmd
    if TRACE:
        try:
            import axon_prof  # noqa: F401  (registers NTFF hook)
        except Exception:
            pass
    res = run_bass_kernel_spmd(nc, in_maps, list(range(N_CORES)), trace=TRACE)
    LAST_RESULT["exec_time_ns"] = res.exec_time_ns
    LAST_RESULT["res"] = res
    return res.results


def kernel(x, edge_index, W_src, W_dst, att, bias, bn_gamma, bn_beta):
    x = np.asarray(x, np.float32)
    edge_index = np.asarray(edge_index)
    prep = _host_prep(x, edge_index, np.asarray(W_src), np.asarray(W_dst),
                      np.asarray(att))

    key = (prep["rounds"],)
    if key not in _PROGRAM_CACHE:
        _PROGRAM_CACHE[key] = _build_program(prep["rounds"], prep["tot"])
    nc = _PROGRAM_CACHE[key]

    in_maps = []
    for k in range(N_CORES):
        in_maps.append({
            "xeT": prep["xeT"][k],
            "xTp": prep["xTp"][k],
            "wsrc": prep["wsrc_bf"],
            "wdst": prep["wdst_bf"],
            "amat": prep["A_bf"],
            "amat2": prep["A2_bf"],
            "idn": prep["ident"],
        })
    results = _run(nc, in_maps)

    out = np.zeros((N_NODES, HC), np.float32)
    for k in range(N_CORES):
        yk = np.asarray(results[k]["y"])[:NODES_PER_CORE]
        yk = yk * prep["cs"].ravel()[None, :]
        out[np.ix_(prep["perms"][k][:NODES_PER_CORE], prep["cperm"])] = yk

    # bias + BatchNorm (batch stats) + LeakyReLU(0.02) epilogue
    out = out + np.asarray(bias, np.float32)[None, :]
    mean = out.mean(axis=0)
    var = out.var(axis=0)
    yv = (np.asarray(bn_gamma, np.float32) * (out - mean)
          / np.sqrt(var + EPS_BN) + np.asarray(bn_beta, np.float32))
    return np.where(yv > 0, yv, 0.02 * yv).astype(np.float32)


# revision 30
# speedup vs baseline: 1.3722x; 1.2343x over previous
"""GATv2 layer on 8 Trainium2 NeuronCores (Bass/Tile).

Self-contained: takes full inputs, shards internally, returns full output.

Strategy (edge-projection, channel-major): edges bucketed by destination
node; each core owns N/8 destinations, degree-sorted into blocks of 128
(one node per grid column). The host pre-gathers x[src] for every edge
slot into a per-core [128ch, slots] bf16 stream, so the device never does
an indirect gather: a W-stationary matmul projects edge slots straight
into channel-major PSUM chunks (t = W_ext^T xe). s = t + h_dst via a
broadcast add; LeakyReLU logits use the identity
a^T LR(s) = sum_pos LR(|a|s) - sum_neg LR(|a|s) with |a| folded into
W_ext, evaluated as a +-1 head-mask matmul (replicated across partitions
so exp runs full-width). den/num come from strided free-axis reduces;
num = sum ex*s - den*h_dst recovers the h_src-weighted sum. Sentinel
slots stream a host-solved x column whose projection makes every head's
logit ~ -2e8, so exp underflows to exactly 0. Softmax max-subtraction is
dropped (mathematically invariant; logits are O(1)).
"""
import os
import sys

for _p in ("/opt/trn_rl_repo", "/root/.axon_site/_ro/trn_rl_repo"):
    if os.path.isdir(_p) and _p not in sys.path:
        sys.path.insert(0, _p)

import numpy as np
import ml_dtypes
import concourse.bass as bass
import concourse.bacc as bacc
import concourse.mybir as mybir
import concourse.tile as tile

P = 128
HEADS = 4
OUT_CH = 32
HC = HEADS * OUT_CH          # 128
EPS_BN = 1e-5
CHUNK = 512                  # PSUM bank = 512 fp32

N_NODES = int(os.environ.get("GAT_N", 100000))
N_CORES = int(os.environ.get("GAT_CORES", 8))
R_CAP = int(os.environ.get("GAT_RCAP", 24))   # multiple of 4
RUN_MODE = os.environ.get("GAT_RUN", "hw")    # hw | sim
# HW Lrelu ignores alpha (fixed 0.01 slope) -- keep LeakyReLU on DVE
USE_ACT_LRELU = RUN_MODE != "sim" and os.environ.get("GAT_LRELU", "0") == "1"
TRACE = os.environ.get("GAT_TRACE", "0") == "1"

NODES_PER_CORE = N_NODES // N_CORES
BLOCKS = (NODES_PER_CORE + P - 1) // P
NPAD = BLOCKS * P

f32 = mybir.dt.float32
bf16 = mybir.dt.bfloat16
bfnp = ml_dtypes.bfloat16

LAST_RESULT = {}
_PROGRAM_CACHE = {}


def _host_prep(x, edge_index, W_src, W_dst, att):
    src = edge_index[0].astype(np.int64)
    dst = edge_index[1].astype(np.int64)
    loop = np.arange(N_NODES, dtype=np.int64)
    src2 = np.concatenate([src, loop])
    dst2 = np.concatenate([dst, loop])
    deg = np.bincount(dst2, minlength=N_NODES)
    order = np.argsort(dst2, kind="stable")
    src_sorted = src2[order].astype(np.int64)
    starts = np.zeros(N_NODES + 1, np.int64)
    starts[1:] = np.cumsum(deg)

    # per-core degree-sorted node permutation (pads replicate the core's
    # first node but get a single self-slot)
    perms = np.zeros((N_CORES, NPAD), np.int64)
    is_pad = np.zeros((N_CORES, NPAD), bool)
    for k in range(N_CORES):
        nodes = np.arange(k * NODES_PER_CORE, (k + 1) * NODES_PER_CORE)
        o = np.argsort(-deg[nodes], kind="stable")
        perms[k, :NODES_PER_CORE] = nodes[o]
        perms[k, NODES_PER_CORE:] = nodes[0]
        is_pad[k, NODES_PER_CORE:] = True

    degp = deg[perms]
    degp[is_pad] = 1
    degb = degp.reshape(N_CORES, BLOCKS, P)
    Rb = degb.max(axis=(0, 2)).astype(np.int64)   # uniform across cores
    nh = int(os.environ.get("GAT_HALVE", "1"))
    if nh >= 2:
        Rb = (Rb + 3) & ~3                        # two halvings: rr % 4 == 0
    elif nh == 1:
        Rb = (Rb + 1) & ~1                        # one halving: rr % 2 == 0

    rounds = []                                   # (block, r_off, rr)
    for b in range(BLOCKS):
        r, roff = int(Rb[b]), 0
        while r > 0:
            rr = min(r, R_CAP)
            rounds.append((b, roff, rr))
            roff += rr
            r -= rr
    tot = sum(rr for _, _, rr in rounds)

    # per-slot source node (SENT = N_NODES -> sentinel row of x_ext),
    # node-major within each round: column = n*rr + r
    SENT = N_NODES
    vals_all = np.full((N_CORES, tot * P), SENT, np.int64)
    off = 0
    for (b, roff, rr) in rounds:
        for k in range(N_CORES):
            nodes = perms[k, b * P:(b + 1) * P]
            pad = is_pad[k, b * P:(b + 1) * P]
            nd = degp[k, b * P:(b + 1) * P]
            j = roff + np.arange(rr)[None, :]                  # [1, rr]
            base = np.where(pad, 0, starts[nodes])[:, None]
            gidx = np.clip(base + j, 0, src_sorted.size - 1)
            v = src_sorted[gidx]                               # [P, rr]
            v = np.where(j < nd[:, None], v, SENT)
            v = np.where(pad[:, None] & (j == 0), nodes[:, None], v)
            vals_all[k, off * P:(off + rr) * P] = v.reshape(-1)
        off += rr

    # --- weights: channel perm (pos att first), |att| prescale ---
    att64 = att.astype(np.float64)
    cperm = np.zeros(HC, np.int64)
    scale = np.zeros(HC, np.float64)
    sbb = []
    for h in range(HEADS):
        pos = np.where(att64[h] > 0)[0]
        neg = np.where(att64[h] <= 0)[0]
        o = np.concatenate([pos, neg])
        sbb.append(len(pos))
        cperm[h * OUT_CH:(h + 1) * OUT_CH] = h * OUT_CH + o
        scale[h * OUT_CH:(h + 1) * OUT_CH] = np.abs(att64[h][o])
    scale = np.maximum(scale, 1e-20)

    def wext(W):
        return (W.astype(np.float64)[:, cperm] * scale[None, :])

    wsrc64 = wext(W_src)
    wdst64 = wext(W_dst)
    wsrc_bf = wsrc64.astype(bfnp)
    wdst_bf = wdst64.astype(bfnp)
    chanscale = (1.0 / scale).astype(np.float32).reshape(HC, 1)

    # logit head-mask matrix, replicated to all 128 output partitions:
    # out channel c' (head h' = (c'//32)): +1 for pos channels of h',
    # -1 for neg channels of h'.
    A = np.zeros((HC, HC), np.float64)
    for h in range(HEADS):
        cs0, cs1 = h * OUT_CH, (h + 1) * OUT_CH
        A[cs0:cs0 + sbb[h], cs0:cs1] = 1.0
        A[cs0 + sbb[h]:cs1, cs0:cs1] = -1.0
    A_bf = A.astype(bfnp)

    # sentinel x column: projects (through the bf16 weights) to
    # t ~ -B*signvec, making every head's logit deeply negative so
    # exp underflows to exactly 0. Verified on the bf16-rounded vector;
    # falls back to a jittered W-range direction if the solve is too
    # ill-conditioned for bf16.
    B = 1e4
    signvec = np.where(A[:, ::OUT_CH].sum(axis=1) > 0, 1.0, -1.0)  # +1 pos
    Wr = wsrc_bf.astype(np.float64)

    def sent_logit(v):
        t = v.astype(bfnp).astype(np.float64) @ Wr
        u = np.maximum(t, 0.2 * t)
        return (u @ A).max()

    cands = [np.linalg.solve(Wr.T, -B * signvec)]
    rng = np.random.default_rng(0)
    for _ in range(20):
        jit = signvec + 0.3 * rng.standard_normal(HC)
        v = Wr @ jit
        cands.append(-B * v / (np.abs(Wr.T @ v).mean() + 1e-30))
    xe_sent = None
    for v in cands:
        if sent_logit(v) < -5e3:
            xe_sent = v
            break
    assert xe_sent is not None, "no robust sentinel direction found"

    x_ext = np.concatenate([np.asarray(x, np.float32),
                            xe_sent[None, :].astype(np.float32)], axis=0)
    x_bf = x_ext.astype(bfnp)

    # per-core channel-major edge stream [128, tot*P]
    xeT = np.empty((N_CORES, P, tot * P), bfnp)
    for k in range(N_CORES):
        xeT[k] = x_bf[vals_all[k]].T

    # per-core dst-node stream [128, NPAD]
    xTp = np.empty((N_CORES, P, NPAD), bfnp)
    for k in range(N_CORES):
        xTp[k] = x_bf[perms[k]].T

    ident = np.eye(P, dtype=np.float32)

    return dict(rounds=tuple(rounds), sbb=tuple(sbb), tot=tot,
                perms=perms, cperm=cperm,
                wsrc_bf=np.ascontiguousarray(wsrc_bf),
                wdst_bf=np.ascontiguousarray(wdst_bf),
                A_bf=np.ascontiguousarray(A_bf),
                cs=chanscale, ident=ident, xeT=xeT, xTp=xTp)


def _build_program(rounds, tot):
    nc = bacc.Bacc("TRN2", target_bir_lowering=False, debug=False,
                   num_devices=N_CORES)
    xeT = nc.dram_tensor("xeT", [P, tot * P], bf16, kind="ExternalInput")
    xTp = nc.dram_tensor("xTp", [P, NPAD], bf16, kind="ExternalInput")
    wsrc = nc.dram_tensor("wsrc", [P, HC], bf16, kind="ExternalInput")
    wdst = nc.dram_tensor("wdst", [P, HC], bf16, kind="ExternalInput")
    amat = nc.dram_tensor("amat", [P, HC], bf16, kind="ExternalInput")
    idn = nc.dram_tensor("idn", [P, P], f32, kind="ExternalInput")
    y = nc.dram_tensor("y", [NPAD, HC], f32, kind="ExternalOutput")

    AX = mybir.AxisListType.X
    OP = mybir.AluOpType
    AF = mybir.ActivationFunctionType

    with tile.TileContext(nc) as tc:
        with (
            tc.tile_pool(name="consts", bufs=1) as cp,
            tc.tile_pool(name="edge", bufs=3) as ep,
            tc.tile_pool(name="work", bufs=4) as wp,
            tc.tile_pool(name="work2", bufs=2) as wp2,
            tc.tile_pool(name="acc", bufs=2) as ap_,
            tc.tile_pool(name="fin", bufs=2) as fp_,
            tc.tile_pool(name="pproj", bufs=3, space="PSUM") as ppj,
            tc.tile_pool(name="plogit", bufs=3, space="PSUM") as plg,
            tc.tile_pool(name="ptrans", bufs=1, space="PSUM") as ptr,
        ):
            wsrc_t = cp.tile([P, HC], bf16)
            nc.sync.dma_start(out=wsrc_t[:], in_=wsrc[:])
            wdst_t = cp.tile([P, HC], bf16)
            nc.sync.dma_start(out=wdst_t[:], in_=wdst[:])
            amat_t = cp.tile([P, HC], bf16)
            nc.sync.dma_start(out=amat_t[:], in_=amat[:])
            idn_t = cp.tile([P, P], f32)
            nc.sync.dma_start(out=idn_t[:], in_=idn[:])
            xtp_t = cp.tile([P, NPAD], bf16)
            nc.sync.dma_start(out=xtp_t[:], in_=xTp[:])

            # ---- edge phase ----
            # (h_dst for the num correction is projected on demand per
            # block-final: one 128-col matmul each, keeping SBUF free.)
            n_in_block = {}
            for b, _, _ in rounds:
                n_in_block[b] = n_in_block.get(b, 0) + 1
            done_in_block = 0
            cur_b = -1
            nd_t = None
            off = 0

            for (b, roff, rr) in rounds:
                first = b != cur_b
                if first:
                    cur_b = b
                    done_in_block = 0
                    nd_t = ap_.tile([P, 2 * P], f32, tag="nd")
                done_in_block += 1
                last = done_in_block == n_in_block[b]

                ns = rr * P
                kn = CHUNK // rr            # nodes per proj chunk

                xet = ep.tile([P, R_CAP * P], bf16, tag="xet")
                nc.sync.dma_start(out=xet[:, :ns],
                                  in_=xeT[:, off * P:(off + rr) * P])
                off += rr

                # projection: s = Wsrc^T xe + Wdst^T xd (0-stride rhs
                # replicates each dst column rr times); ACT drains PSUM
                s_t = wp.tile([P, R_CAP * P], bf16, tag="s")
                n0 = 0
                while n0 < P:
                    k = min(kn, P - n0)
                    c0, cw = n0 * rr, k * rr
                    ps = ppj.tile([P, CHUNK], f32, space="PSUM", tag="pp")
                    nc.tensor.matmul(out=ps[:, :cw], lhsT=wsrc_t[:],
                                     rhs=xet[:, c0:c0 + cw],
                                     start=True, stop=False)
                    a = xtp_t[:, b * P + n0:b * P + n0 + k]
                    xdv = bass.AP(a.tensor, a.offset,
                                  [list(a.ap[0]), list(a.ap[-1]), [0, rr]])
                    nc.tensor.matmul(out=ps[:, :cw], lhsT=wdst_t[:],
                                     rhs=xdv, start=False, stop=True)
                    nc.scalar.copy(out=s_t[:, c0:c0 + cw], in_=ps[:, :cw])
                    n0 += k

                # LeakyReLU as 4x-mode tensor_scalar (0.2*s) + 2x-mode
                # tensor_tensor max -- STT runs at 1x, this pair doesn't.
                u_t = wp.tile([P, R_CAP * P], bf16, tag="u")
                nc.vector.tensor_scalar(out=u_t[:, :ns], in0=s_t[:, :ns],
                                        scalar1=0.2, scalar2=None,
                                        op0=OP.mult)
                nc.vector.tensor_tensor(out=u_t[:, :ns], in0=s_t[:, :ns],
                                        in1=u_t[:, :ns], op=OP.max)

                em_t = wp.tile([P, 2 * R_CAP * P], bf16, tag="em")
                for c0 in range(0, ns, CHUNK):
                    cw = min(CHUNK, ns - c0)
                    pl = plg.tile([P, CHUNK], f32, space="PSUM", tag="pl")
                    nc.tensor.matmul(out=pl[:, :cw], lhsT=amat_t[:],
                                     rhs=u_t[:, c0:c0 + cw],
                                     start=True, stop=True)
                    nc.scalar.activation(out=em_t[:, c0:c0 + cw],
                                         in_=pl[:, :cw], func=AF.Exp)

                nc.vector.tensor_tensor(out=em_t[:, ns:2 * ns],
                                        in0=em_t[:, :ns],
                                        in1=s_t[:, :ns], op=OP.mult)

                # segmented sum over r: bf16 pair-halvings (2x-mode TT;
                # one for rr%4==2, two for rr%4==0) then a short strided
                # reduce (reduce has no fast DVE modes, so shrink its
                # input as far as cheap TTs allow).
                target = nd_t if first else wp2.tile([P, 2 * P], f32,
                                                     tag="ndt")
                tv = target[:].rearrange("p (g n) -> p g n", g=2)
                tv1 = target[:].rearrange("p (g n r) -> p g n r", g=2, r=1)
                emv0 = em_t[:, :2 * ns].rearrange("p (g n r) -> p g n r",
                                                  g=2, r=rr)
                if rr == 2:
                    nc.vector.tensor_tensor(out=tv1, in0=emv0[:, :, :, :1],
                                            in1=emv0[:, :, :, 1:],
                                            op=OP.add)
                else:
                    h1 = rr // 2
                    em2 = wp2.tile([P, R_CAP * P], bf16, tag="em2")
                    e2v = em2[:, :2 * P * h1].rearrange(
                        "p (g n r) -> p g n r", g=2, r=h1)
                    nc.vector.tensor_tensor(out=e2v, in0=emv0[:, :, :, :h1],
                                            in1=emv0[:, :, :, h1:],
                                            op=OP.add)
                    if rr % 4 == 0:
                        h2 = rr // 4
                        if h2 == 1:
                            nc.vector.tensor_tensor(out=tv1,
                                                    in0=e2v[:, :, :, :1],
                                                    in1=e2v[:, :, :, 1:],
                                                    op=OP.add)
                        else:
                            nc.vector.tensor_tensor(
                                out=e2v[:, :, :, :h2],
                                in0=e2v[:, :, :, :h2],
                                in1=e2v[:, :, :, h2:2 * h2], op=OP.add)
                            nc.vector.reduce_sum(out=tv,
                                                 in_=e2v[:, :, :, :h2],
                                                 axis=AX)
                    else:
                        nc.vector.reduce_sum(out=tv, in_=e2v, axis=AX)
                if not first:
                    nc.gpsimd.tensor_tensor(out=nd_t[:], in0=nd_t[:],
                                            in1=target[:], op=OP.add)

                if last:
                    # t2 = num - den*hd (Pool); transpose t2, den (PE);
                    # y_nm = t2_nm * recip(den_nm) broadcast per head
                    php = ppj.tile([P, CHUNK], f32, space="PSUM", tag="pp")
                    nc.tensor.matmul(out=php[:, :P], lhsT=wdst_t[:],
                                     rhs=xtp_t[:, b * P:(b + 1) * P],
                                     start=True, stop=True)
                    hd_b = fp_.tile([P, P], f32, tag="hdb")
                    nc.scalar.copy(out=hd_b[:], in_=php[:, :P])
                    t1 = fp_.tile([P, P], f32, tag="t1")
                    nc.gpsimd.tensor_tensor(out=t1[:], in0=nd_t[:, :P],
                                            in1=hd_b[:], op=OP.mult)
                    nc.gpsimd.tensor_tensor(out=t1[:], in0=nd_t[:, P:],
                                            in1=t1[:], op=OP.subtract)
                    pt = ptr.tile([P, P], f32, space="PSUM", tag="pt")
                    nc.tensor.transpose(out=pt[:], in_=t1[:],
                                        identity=idn_t[:])
                    pd = ptr.tile([P, P], f32, space="PSUM", tag="pd")
                    nc.tensor.transpose(out=pd[:], in_=nd_t[:, :P],
                                        identity=idn_t[:])
                    rden = fp_.tile([P, HEADS], f32, tag="rden")
                    dh = pd[:]
                    dhv = bass.AP(dh.tensor, dh.offset,
                                  [list(dh.ap[0]), [OUT_CH, HEADS]])
                    nc.vector.reciprocal(out=rden[:], in_=dhv)
                    ynm = fp_.tile([P, P], f32, tag="ynm")
                    r = rden[:]
                    rv = bass.AP(r.tensor, r.offset,
                                 [list(r.ap[0]), [1, HEADS], [0, OUT_CH]])
                    yv = ynm[:].rearrange("p (h c) -> p h c", c=OUT_CH)
                    pv2 = pt[:].rearrange("p (h c) -> p h c", c=OUT_CH)
                    nc.vector.tensor_tensor(out=yv, in0=pv2, in1=rv,
                                            op=OP.mult)
                    nc.sync.dma_start(out=y[b * P:(b + 1) * P, :],
                                      in_=ynm[:])

    nc.compile()
    return nc


def _run(nc, in_maps):
    if RUN_MODE == "sim":
        from concourse import bass_interp
        assert N_CORES == 1
        sim = bass_interp.CoreSim(nc)
        for name, arr in in_maps[0].items():
            sim.tensor(name)[:] = arr
        sim.simulate()
        return [{"y": np.array(sim.tensor("y"))}]
    from concourse.bass_utils import run_bass_kernel_spmd
    if TRACE:
        try:
            import axon_prof  # noqa: F401  (registers NTFF hook)
        except Exception:
            pass
    res = run_bass_kernel_spmd(nc, in_maps, list(range(N_CORES)), trace=TRACE)
    LAST_RESULT["exec_time_ns"] = res.exec_time_ns
    LAST_RESULT["res"] = res
    return res.results


def kernel(x, edge_index, W_src, W_dst, att, bias, bn_gamma, bn_beta):
    x = np.asarray(x, np.float32)
    edge_index = np.asarray(edge_index)
    prep = _host_prep(x, edge_index, np.asarray(W_src), np.asarray(W_dst),
                      np.asarray(att))

    key = (prep["rounds"],)
    if key not in _PROGRAM_CACHE:
        _PROGRAM_CACHE[key] = _build_program(prep["rounds"], prep["tot"])
    nc = _PROGRAM_CACHE[key]

    in_maps = []
    for k in range(N_CORES):
        in_maps.append({
            "xeT": prep["xeT"][k],
            "xTp": prep["xTp"][k],
            "wsrc": prep["wsrc_bf"],
            "wdst": prep["wdst_bf"],
            "amat": prep["A_bf"],
            "idn": prep["ident"],
        })
    results = _run(nc, in_maps)

    out = np.zeros((N_NODES, HC), np.float32)
    for k in range(N_CORES):
        yk = np.asarray(results[k]["y"])[:NODES_PER_CORE]
        yk = yk * prep["cs"].ravel()[None, :]
        out[np.ix_(prep["perms"][k][:NODES_PER_CORE], prep["cperm"])] = yk

    # bias + BatchNorm (batch stats) + LeakyReLU(0.02) epilogue
    out = out + np.asarray(bias, np.float32)[None, :]
    mean = out.mean(axis=0)
    var = out.var(axis=0)
    yv = (np.asarray(bn_gamma, np.float32) * (out - mean)
          / np.sqrt(var + EPS_BN) + np.asarray(bn_beta, np.float32))
    return np.where(yv > 0, yv, 0.02 * yv).astype(np.float32)

